# revision 23
# baseline (speedup 1.0000x reference)
"""Trainium2 Bass kernel for nn_Attention_16028817948779.

Reference computation (b=4, c=256, heads=8, d=64, h=w=48, n=2304):
  qkv = w_qkv @ x          (1x1 conv)
  q,k,v -> [b, H, d, n];  q,k l2-normalized along n (spatial)
  sim  = (q^T k) * 10;  attn = softmax(sim, axis=-1)
  out  = attn @ v^T -> [b, H, n, d] -> [b, H*d, h, w]
  y    = w_out @ out + b_out
Sharding: 8 cores; core c handles batch c//2, head group (c%2)*4..+4.
Each core computes a partial y over its 4 heads; host sums the two
partials per batch and adds the bias.

Kernel design (v2):
  - Attention math in bf16 (q, k, v^T, exp(sim)); projections' matmuls in
    bf16 with fp32 PSUM accumulation; output projection in f32r.
  - Attention in transposed form ST[j,i] = k_j . q_i; exp without
    max-subtraction (|logits| small because q,k are l2-normalized along n);
    softmax denominator comes free from a ones-column appended to V^T.
  - Phase order: q,k projection first -> l2norm scale chain -> attention,
    with the V^T projection matmuls interleaved into the first attention
    block's j-loop so the ACT exp stream starts as early as possible.
  - Denominator reciprocal via the 1-instruction approx DVE op (the exact
    nc.vector.reciprocal costs ~3.3us per row).
  - Softmax normalize multiply runs on GPSIMD (SBUF-only engine) to keep
    DVE free; DVE handles all PSUM reads.
"""

import os
import sys

import numpy as np

_TRN_REPO = "/opt/trn_rl_repo"
if _TRN_REPO not in sys.path:
    sys.path.insert(0, _TRN_REPO)

B = 4
C = 256
HEADS = 8
D = 64
N = 2304  # 48*48
HID = HEADS * D  # 512

N_CORES = 8
CI = 2  # c chunks of 128
# i/n chunks of <=512 (PSUM bank / fp32 moving-operand limit)
NCHUNKS = [(0, 512), (512, 512), (1024, 512), (1536, 512), (2048, 256)]
NJ = N // 128  # 18 key chunks of 128


def _apply_compat_patches():
    """walrus in this env only accepts ~1 sync wait per instruction, but the
    Tile framework attaches one wait per outstanding proc to a single
    instruction. Split excess waits onto EventSemaphore instructions at the
    BIR-JSON level (Bass.to_json_bytes is the serialization choke point for
    both the native and the axon/PJRT compile paths)."""
    import json

    import concourse.bass as bass

    if getattr(bass.Bass.to_json_bytes, "_waitsplit", False):
        return

    MAXW = 1
    _orig = bass.Bass.to_json_bytes

    def _split_waits(raw):
        m = json.loads(raw)
        ctr = 0
        changed = False
        for f in m.get("functions", []):
            for blk in f.get("blocks", []):
                new_insts = []
                for ins in blk.get("instructions", []):
                    si = ins.get("sync_info")
                    waits = (si or {}).get("on_wait") or []
                    if len(waits) > MAXW:
                        changed = True
                        for w in waits[:-MAXW]:
                            ctr += 1
                            new_insts.append(
                                {
                                    "debug": ins.get("debug", 0),
                                    "engine": ins["engine"],
                                    "ins": [],
                                    "outs": [],
                                    "name": f"waitsplit_{ctr}",
                                    "opcode": "EventSemaphore",
                                    "sync_info": {"on_update": [], "on_wait": [w]},
                                }
                            )
                        si["on_wait"] = waits[-MAXW:]
                    new_insts.append(ins)
                blk["instructions"] = new_insts
        return json.dumps(m).encode() if changed else raw

    def _patched(self):
        return _split_waits(_orig(self))

    _patched._waitsplit = True
    bass.Bass.to_json_bytes = _patched

    if os.environ.get("KERNEL_LDWOPT", "0") == "1":
        import concourse.bass_utils as bu

        if not getattr(bu.run_command, "_ldwopt", False):
            _orig_rc = bu.run_command

            def _rc(cmd, *a, **kw):
                cmd = [
                    c.replace("--enable-ldw-opt=false", "--enable-ldw-opt=true")
                    if isinstance(c, str)
                    else c
                    for c in cmd
                ]
                return _orig_rc(cmd, *a, **kw)

            _rc._ldwopt = True
            bu.run_command = _rc


def build_kernel():
    import concourse.bass as bass
    import concourse.mybir as mybir
    import concourse.tile as tile

    _apply_compat_patches()

    f32 = mybir.dt.float32
    f32r = mybir.dt.float32r
    bf16 = mybir.dt.bfloat16
    i16 = mybir.dt.int16
    Exp = mybir.ActivationFunctionType.Exp
    Ln = mybir.ActivationFunctionType.Ln
    Square = mybir.ActivationFunctionType.Square
    mult = mybir.AluOpType.mult
    add = mybir.AluOpType.add
    X = mybir.AxisListType.X

    # Schraudolph exp on DVE: bf16_bits(e^x) ~= round(x*A16 + B16); the
    # int16 write rounds to nearest, the bf16 bit pattern IS the result.
    # C centers the multiplicative sawtooth error (+-3%).
    A16 = 128.0 / float(np.log(2.0))
    B16 = 127.0 * 128.0 - 5.5
    # every SCHR_MOD-th j-chunk's exp runs on DVE instead of ACT
    SCHR_MOD = 3

    nc = bass.Bass()
    x_d = nc.dram_tensor("x", [C, N], bf16, kind="ExternalInput")
    wqT_d = nc.dram_tensor("wqT", [C, 256], bf16, kind="ExternalInput")
    wkT_d = nc.dram_tensor("wkT", [C, 256], bf16, kind="ExternalInput")
    wvT_d = nc.dram_tensor("wvT", [C, 256], bf16, kind="ExternalInput")
    woutT_d = nc.dram_tensor("woutT", [128, 2, 256], f32r, kind="ExternalInput")
    y_d = nc.dram_tensor("y", [C, N], f32, kind="ExternalOutput")

    with tile.TileContext(nc) as tc:
        with (
            tc.tile_pool(name="persist", bufs=1) as pp,
            tc.tile_pool(name="pt", bufs=4) as ptp,
            tc.tile_pool(name="misc", bufs=2) as mp,
            tc.tile_pool(name="dram", bufs=4, space="DRAM") as dp,
            tc.tile_pool(name="ps_st", bufs=3, space="PSUM") as ps_st,
            tc.tile_pool(name="ps_pv", bufs=2, space="PSUM") as ps_pv,
        ):
            # ---- load inputs ----
            x_sb = pp.tile([128, CI, N], bf16)
            for ci in range(CI):
                for ns, nl in NCHUNKS:
                    nc.sync.dma_start(
                        out=x_sb[:, ci, ns : ns + nl],
                        in_=x_d[ci * 128 : (ci + 1) * 128, ns : ns + nl],
                    )
            wq_sb = pp.tile([128, CI, 256], bf16)
            wk_sb = pp.tile([128, CI, 256], bf16)
            wv_sb = pp.tile([128, CI, 256], bf16)
            for w_sb, w_d in ((wq_sb, wqT_d), (wk_sb, wkT_d), (wv_sb, wvT_d)):
                nc.sync.dma_start(
                    out=w_sb[:], in_=w_d.rearrange("(ci p) o -> p ci o", p=128)
                )
            wo_sb = pp.tile([128, 2, 256], f32r)
            nc.sync.dma_start(out=wo_sb[:], in_=woutT_d[:])

            ones_f = pp.tile([128, 1], f32)
            nc.vector.memset(ones_f[:], 1.0)

            # PE warm-up: dummy bf16 matmuls with no input dependencies,
            # executed during the initial DMA wait so the PE p-state is at
            # full speed when the real QKV matmuls arrive.
            warm_sb = pp.tile([128, 512], bf16)
            nc.vector.memset(warm_sb[:], 1.0)
            warm_ps = ps_st.tile([128, 2, 512], f32, tag="st", name="warm_ps")
            for wi in range(32):
                nc.tensor.matmul(
                    warm_ps[:, 0, :],
                    lhsT=warm_sb[:, 0:128],
                    rhs=warm_sb[:],
                    start=(wi == 0),
                    stop=(wi == 31),
                )
            nc.vector.tensor_copy(warm_sb[:, 0:16], warm_ps[:, 0, 0:16])

            # vt_sb: [n-part, j-chunk, 4*65]; per head 64 v columns + ones col
            # (filled during the first attention block)
            vt_sb = pp.tile([128, NJ, 260], bf16)
            vt4 = vt_sb.rearrange("p j (h e) -> p j h e", e=65)
            with nc.allow_low_precision(reason="ones column in bf16"):
                nc.vector.tensor_copy(
                    vt4[:, :, :, 64:65],
                    ones_f[:, 0:1]
                    .unsqueeze(1)
                    .unsqueeze(1)
                    .to_broadcast((128, NJ, 4, 1)),
                )

            def emit_vt(j):
                # V^T projection chunk j -> vt_sb (bf16)
                vps3 = ps_st.tile([128, 2, 512], f32, tag="st", name="v_ps")
                vps = vps3[:, 0, 0:256]
                for ci in range(CI):
                    nc.tensor.matmul(
                        vps[:],
                        lhsT=x_sb[:, ci, j * 128 : (j + 1) * 128],
                        rhs=wv_sb[:, ci, :],
                        start=(ci == 0),
                        stop=(ci == CI - 1),
                    )
                with nc.allow_low_precision(reason="v^T stored bf16"):
                    nc.vector.tensor_copy(
                        vt4[:, j, :, 0:64],
                        vps.rearrange("p (h d) -> p h d", h=4),
                    )

            # First half of the V^T projection: keeps the PE warm while the
            # x/w DMAs for q/k drain, and its DVE copies run before the q/k
            # PSUM casts.
            VT_SPLIT = 9
            for j in range(VT_SPLIT):
                emit_vt(j)

            # ---- Q/K projection ----
            # q_sb/k_sb: [d-part, head-pair, n]; heads 2p at part 0-63,
            # 2p+1 at 64-127
            q_sb = pp.tile([128, 2, N], bf16)
            k_sb = pp.tile([128, 2, N], bf16)
            ssq = mp.tile([128, 2, 2, len(NCHUNKS)], f32, tag="ssq")
            scratch = pp.tile([128, 512], f32)
            with nc.allow_low_precision(reason="q/k stored bf16"):
                for ti, (dst, w_sb) in enumerate(((q_sb, wq_sb), (k_sb, wk_sb))):
                    for oc in range(2):
                        for nci, (ns, nl) in enumerate(NCHUNKS):
                            ps3 = ps_st.tile([128, 2, 512], f32, tag="st", name="qk_ps")
                            ps = ps3[:, 0, :]
                            for ci in range(CI):
                                nc.tensor.matmul(
                                    ps[:, :nl],
                                    lhsT=w_sb[:, ci, oc * 128 : (oc + 1) * 128],
                                    rhs=x_sb[:, ci, ns : ns + nl],
                                    start=(ci == 0),
                                    stop=(ci == CI - 1),
                                )
                            nc.vector.tensor_copy(dst[:, oc, ns : ns + nl], ps[:, :nl])
                            nc.scalar.activation(
                                scratch[:, :nl],
                                ps[:, :nl],
                                Square,
                                accum_out=ssq[:, ti, oc, nci : nci + 1],
                            )

            # ---- fold l2norm + SCALE into q: q *= 10/sqrt(ssq_q*ssq_k) ----
            sqk = mp.tile([128, 2, 2], f32, tag="sqk")
            nc.vector.reduce_sum(
                sqk.rearrange("p a b -> p (a b)"),
                ssq.rearrange("p a b c -> p (a b) c"),
                axis=X,
            )
            qscale = mp.tile([128, 2], f32, tag="qscale")
            nc.vector.tensor_tensor(qscale[:], sqk[:, 0, :], sqk[:, 1, :], mult)
            # 10/sqrt(x) = exp(-0.5*ln(x) + ln(10)); Ln and Exp share one ACT
            # table set, so no extra table load next to the softmax exps
            nc.scalar.activation(qscale[:], qscale[:], Ln)
            ln10 = mp.tile([128, 1], f32, tag="ln10")
            nc.vector.memset(ln10[:], 2.302585092994046)
            nc.scalar.activation(qscale[:], qscale[:], Exp, bias=ln10[:], scale=-0.5)

            with nc.allow_low_precision(reason="q scale written as bf16"):
                # chunk-split so the first ST matmuls only wait on chunk 0;
                # emitted BEFORE the remaining vt copies so the in-order DVE
                # unblocks the attention start first
                for ns, nl in NCHUNKS:
                    for oc in range(2):
                        nc.vector.tensor_scalar_mul(
                            q_sb[:, oc, ns : ns + nl],
                            q_sb[:, oc, ns : ns + nl],
                            qscale[:, oc : oc + 1],
                        )

            # Second half of the V^T projection: fills the PE while the
            # l2norm chain and q-scaling run on ACT/DVE, so the PE never
            # idles (an idle PE triggers HAM down-clocking right at the
            # attention start).
            for j in range(VT_SPLIT, NJ):
                emit_vt(j)

            # ---- attention per head pair p (local heads 2p, 2p+1) ----
            # outT_pair[p]: heads 2p / 2p+1 at partitions 0-63 / 64-127, so
            # the output projection contracts K=128 in one matmul per chunk.
            outT = [
                pp.tile([128, N], f32r, name=f"outT{p}", tag=f"outT{p}")
                for p in range(2)
            ]

            def emit_proj(ns, il):
                for oc_ in range(2):
                    yps = ps_pv.tile([128, 512], f32, tag="pv", name="yps")
                    for pr in range(2):
                        nc.tensor.matmul(
                            yps[:, :il],
                            lhsT=wo_sb[:, pr, oc_ * 128 : (oc_ + 1) * 128],
                            rhs=outT[pr][:, ns : ns + il],
                            start=(pr == 0),
                            stop=(pr == 1),
                        )
                    y_sb = mp.tile([128, 512], f32, tag="ysb", name="y_sb")
                    nc.scalar.copy(y_sb[:, :il], yps[:, :il])
                    nc.sync.dma_start(
                        out=y_d[oc_ * 128 : (oc_ + 1) * 128, ns : ns + il],
                        in_=y_sb[:, :il],
                    )

            # Flattened attention stream over blocks (p, nci) with a GLOBAL
            # one-step PV lag: the last PV pair of a block is emitted after
            # the first ST pair of the next block, so the PE never waits for
            # an exp at block boundaries. Each block's PSUM accumulators are
            # allocated lazily on the first PV so only two are ever live.
            class Blk:
                def __init__(self, p, ns, il):
                    self.p, self.ns, self.il = p, ns, il
                    self.pv = None

                def emit_pv(self, pt_, j_):
                    if self.pv is None:
                        self.pv = (
                            ps_pv.tile([65, 512], f32, tag="pv", name="pvA"),
                            ps_pv.tile([65, 512], f32, tag="pv", name="pvB"),
                        )
                    for t in range(2):
                        nc.tensor.matmul(
                            self.pv[t][:, : self.il],
                            lhsT=vt4[:, j_, 2 * self.p + t, :],
                            rhs=pt_[:, t, : self.il],
                            start=(j_ == 0),
                            stop=(j_ == NJ - 1),
                        )

                def emit_tail(self):
                    # Free the PV accumulators into SBUF, then normalize rows
                    # 0-63 by row 64 (softmax denominator): the two [1,il]
                    # denominator rows round-trip through DRAM packed as
                    # [128, 2*il/128] so ONE tiny exact reciprocal covers the
                    # whole block, then partition-replicated back.
                    p, ns, il = self.p, self.ns, self.il
                    nsbs = []
                    for pv in self.pv:
                        nsb = mp.tile([65, 512], f32, tag="nsb", name="nsb", bufs=4)
                        nc.scalar.copy(nsb[:, :il], pv[:, :il])
                        nsbs.append(nsb)
                    den_d = dp.tile([1, 1024], f32, tag="dend", name="den_d")
                    for idx, nsb in enumerate(nsbs):
                        nc.sync.dma_start(
                            out=den_d[0:1, idx * il : (idx + 1) * il],
                            in_=nsb[64:65, :il],
                        )
                    g = (2 * il) // 128
                    denp = mp.tile([128, 8], f32, tag="denp", name="denp", bufs=4)
                    nc.sync.dma_start(
                        out=denp[:, :g],
                        in_=den_d.rearrange("o (pp f) -> (o pp) f", f=g)[0:128, :],
                    )
                    nc.vector.reciprocal(denp[:, :g], denp[:, :g])
                    rec_d = dp.tile([1, 1024], f32, tag="recd", name="rec_d")
                    nc.sync.dma_start(
                        out=rec_d.rearrange("o (pp f) -> (o pp) f", f=g)[0:128, :],
                        in_=denp[:, :g],
                    )
                    for idx, nsb in enumerate(nsbs):
                        bc_sb = mp.tile(
                            [64, 512], f32, tag="bcsb", name="bc_sb", bufs=4
                        )
                        nc.sync.dma_start(
                            out=bc_sb[:, :il],
                            in_=rec_d[0:1, idx * il : (idx + 1) * il].to_broadcast(
                                (64, il)
                            ),
                        )
                        nc.gpsimd.tensor_tensor(
                            outT[p][idx * 64 : (idx + 1) * 64, ns : ns + il],
                            nsb[0:64, :il],
                            bc_sb[:, :il],
                            mult,
                        )

            blocks = [
                Blk(p, ns, il) for p in range(2) for (ns, il) in NCHUNKS
            ]
            pv_q = []  # PV pairs lagging 2 STs behind (exp fully drained)
            pending_proj = None  # (ns, il) deferred one extra block so the
            # proj matmuls never wait on a fresh normalize chain

            def flush_pv(limit):
                nonlocal pending_proj
                while len(pv_q) > limit:
                    b_, pt_, j_ = pv_q.pop(0)
                    b_.emit_pv(pt_, j_)
                    if j_ == NJ - 1:
                        b_.emit_tail()
                        if pending_proj is not None:
                            emit_proj(*pending_proj)
                            pending_proj = None
                        if b_.p == 1:
                            pending_proj = (b_.ns, b_.il)
            for bi, blk in enumerate(blocks):
                p, ns, il = blk.p, blk.ns, blk.il
                for j in range(NJ):
                    st = ps_st.tile([128, 2, 512], f32, tag="st", name="st")
                    nc.tensor.matmul(
                        st[:, 0, :il],
                        lhsT=k_sb[0:64, p, j * 128 : (j + 1) * 128],
                        rhs=q_sb[0:64, p, ns : ns + il],
                    )
                    nc.tensor.matmul(
                        st[:, 1, :il],
                        lhsT=k_sb[64:128, p, j * 128 : (j + 1) * 128],
                        rhs=q_sb[64:128, p, ns : ns + il],
                    )
                    pt = ptp.tile([128, 2, 512], bf16, tag="pt", name="pt")
                    if j % SCHR_MOD == SCHR_MOD - 1:
                        with nc.allow_low_precision(reason="schraudolph exp bits"):
                            nc.vector.tensor_scalar(
                                out=pt[:, :, :il].bitcast(i16),
                                in0=st[:, :, :il],
                                scalar1=A16,
                                scalar2=B16,
                                op0=mult,
                                op1=add,
                            )
                    else:
                        nc.scalar.activation(pt[:, :, :il], st[:, :, :il], Exp)
                    flush_pv(2)
                    pv_q.append((blk, pt, j))

            flush_pv(0)
            if pending_proj is not None:
                emit_proj(*pending_proj)

    return nc


_NC_CACHE = None


def kernel(x, w_qkv, w_out, b_out):
    global _NC_CACHE
    import ml_dtypes
    from concourse.bass_utils import run_bass_kernel_spmd

    bf = ml_dtypes.bfloat16
    x = np.ascontiguousarray(x, dtype=np.float32)
    w_qkv = np.asarray(w_qkv, dtype=np.float32)
    w_out = np.asarray(w_out, dtype=np.float32)
    b_out = np.asarray(b_out, dtype=np.float32)

    b, c, h, w = x.shape
    assert (b, c, h, w) == (B, C, 48, 48)
    x_bn = x.reshape(B, C, N).astype(bf)

    wq, wk, wv = w_qkv[0:HID], w_qkv[HID : 2 * HID], w_qkv[2 * HID : 3 * HID]
    w_outT = np.ascontiguousarray(w_out.T)  # [HID, C]

    in_maps = []
    for core in range(N_CORES):
        bb, g = core // 2, core % 2
        rows = slice(g * 256, g * 256 + 256)
        woutT_c = np.ascontiguousarray(
            w_outT[rows].reshape(2, 128, 256).transpose(1, 0, 2)
        )
        in_maps.append(
            {
                "x": np.ascontiguousarray(x_bn[bb]),
                "wqT": np.ascontiguousarray(wq[rows].T.astype(bf)),
                "wkT": np.ascontiguousarray(wk[rows].T.astype(bf)),
                "wvT": np.ascontiguousarray(wv[rows].T.astype(bf)),
                "woutT": woutT_c,
            }
        )

    if _NC_CACHE is None:
        _NC_CACHE = build_kernel()
    nc = _NC_CACHE

    trace = bool(int(os.environ.get("KERNEL_TRACE", "0")))
    res = run_bass_kernel_spmd(
        nc,
        in_maps,
        core_ids=list(range(N_CORES)),
        trace=trace,
        trace_cores=list(range(N_CORES)) if trace else None,
    )
    kernel.last_result = res

    y = np.empty((B, C, N), dtype=np.float32)
    for bb in range(B):
        y[bb] = (
            res.results[2 * bb]["y"]
            + res.results[2 * bb + 1]["y"]
            + b_out[:, None]
        )
    return y.reshape(B, C, 48, 48)


# revision 26
# speedup vs baseline: 1.1441x; 1.1441x over previous
"""Trainium2 Bass kernel for nn_Attention_16028817948779.

Reference computation (b=4, c=256, heads=8, d=64, h=w=48, n=2304):
  qkv = w_qkv @ x          (1x1 conv)
  q,k,v -> [b, H, d, n];  q,k l2-normalized along n (spatial)
  sim  = (q^T k) * 10;  attn = softmax(sim, axis=-1)
  out  = attn @ v^T -> [b, H, n, d] -> [b, H*d, h, w]
  y    = w_out @ out + b_out
Sharding: 8 cores; core c handles batch c//2, head group (c%2)*4..+4.
Each core computes a partial y over its 4 heads; host sums the two
partials per batch and adds the bias.

Kernel design (v2):
  - Attention math in bf16 (q, k, v^T, exp(sim)); projections' matmuls in
    bf16 with fp32 PSUM accumulation; output projection in f32r.
  - Attention in transposed form ST[j,i] = k_j . q_i; exp without
    max-subtraction (|logits| small because q,k are l2-normalized along n);
    softmax denominator comes free from a ones-column appended to V^T.
  - Phase order: q,k projection first -> l2norm scale chain -> attention,
    with the V^T projection matmuls interleaved into the first attention
    block's j-loop so the ACT exp stream starts as early as possible.
  - Denominator reciprocal via the 1-instruction approx DVE op (the exact
    nc.vector.reciprocal costs ~3.3us per row).
  - Softmax normalize multiply runs on GPSIMD (SBUF-only engine) to keep
    DVE free; DVE handles all PSUM reads.
"""

import os
import sys

import numpy as np

_TRN_REPO = "/opt/trn_rl_repo"
if _TRN_REPO not in sys.path:
    sys.path.insert(0, _TRN_REPO)

B = 4
C = 256
HEADS = 8
D = 64
N = 2304  # 48*48
HID = HEADS * D  # 512

N_CORES = 8
CI = 2  # c chunks of 128
# i/n chunks of <=512 (PSUM bank / fp32 moving-operand limit)
NCHUNKS = [(0, 512), (512, 512), (1024, 512), (1536, 512), (2048, 256)]
NJ = N // 128  # 18 key chunks of 128


def _apply_compat_patches():
    """walrus in this env only accepts ~1 sync wait per instruction, but the
    Tile framework attaches one wait per outstanding proc to a single
    instruction. Split excess waits onto EventSemaphore instructions at the
    BIR-JSON level (Bass.to_json_bytes is the serialization choke point for
    both the native and the axon/PJRT compile paths)."""
    import json

    import concourse.bass as bass

    if getattr(bass.Bass.to_json_bytes, "_waitsplit", False):
        return

    MAXW = 1
    _orig = bass.Bass.to_json_bytes

    def _split_waits(raw):
        m = json.loads(raw)
        ctr = 0
        changed = False
        for f in m.get("functions", []):
            for blk in f.get("blocks", []):
                new_insts = []
                for ins in blk.get("instructions", []):
                    si = ins.get("sync_info")
                    waits = (si or {}).get("on_wait") or []
                    if len(waits) > MAXW:
                        changed = True
                        for w in waits[:-MAXW]:
                            ctr += 1
                            new_insts.append(
                                {
                                    "debug": ins.get("debug", 0),
                                    "engine": ins["engine"],
                                    "ins": [],
                                    "outs": [],
                                    "name": f"waitsplit_{ctr}",
                                    "opcode": "EventSemaphore",
                                    "sync_info": {"on_update": [], "on_wait": [w]},
                                }
                            )
                        si["on_wait"] = waits[-MAXW:]
                    new_insts.append(ins)
                blk["instructions"] = new_insts
        return json.dumps(m).encode() if changed else raw

    def _patched(self):
        return _split_waits(_orig(self))

    _patched._waitsplit = True
    bass.Bass.to_json_bytes = _patched

    if os.environ.get("KERNEL_LDWOPT", "0") == "1":
        import concourse.bass_utils as bu

        if not getattr(bu.run_command, "_ldwopt", False):
            _orig_rc = bu.run_command

            def _rc(cmd, *a, **kw):
                cmd = [
                    c.replace("--enable-ldw-opt=false", "--enable-ldw-opt=true")
                    if isinstance(c, str)
                    else c
                    for c in cmd
                ]
                return _orig_rc(cmd, *a, **kw)

            _rc._ldwopt = True
            bu.run_command = _rc


def build_kernel():
    import concourse.bass as bass
    import concourse.mybir as mybir
    import concourse.tile as tile

    _apply_compat_patches()

    f32 = mybir.dt.float32
    f32r = mybir.dt.float32r
    bf16 = mybir.dt.bfloat16
    i16 = mybir.dt.int16
    Exp = mybir.ActivationFunctionType.Exp
    Ln = mybir.ActivationFunctionType.Ln
    Square = mybir.ActivationFunctionType.Square
    mult = mybir.AluOpType.mult
    add = mybir.AluOpType.add
    X = mybir.AxisListType.X

    # Schraudolph exp on DVE: bf16_bits(e^x) ~= round(x*A16 + B16); the
    # int16 write rounds to nearest, the bf16 bit pattern IS the result.
    # C centers the multiplicative sawtooth error (+-3%).
    A16 = 128.0 / float(np.log(2.0))
    B16 = 127.0 * 128.0 - 5.5
    # every SCHR_MOD-th j-chunk's exp runs on DVE instead of ACT
    SCHR_MOD = 3

    nc = bass.Bass()
    x_d = nc.dram_tensor("x", [C, N], bf16, kind="ExternalInput")
    wqT_d = nc.dram_tensor("wqT", [C, 256], bf16, kind="ExternalInput")
    wkT_d = nc.dram_tensor("wkT", [C, 256], bf16, kind="ExternalInput")
    wvT_d = nc.dram_tensor("wvT", [C, 256], bf16, kind="ExternalInput")
    woutT_d = nc.dram_tensor("woutT", [128, 2, 256], f32r, kind="ExternalInput")
    y_d = nc.dram_tensor("y", [C, N], f32, kind="ExternalOutput")

    with tile.TileContext(nc) as tc:
        with (
            tc.tile_pool(name="persist", bufs=1) as pp,
            tc.tile_pool(name="pt", bufs=4) as ptp,
            tc.tile_pool(name="misc", bufs=2) as mp,
            tc.tile_pool(name="dram", bufs=4, space="DRAM") as dp,
            tc.tile_pool(name="ps_st", bufs=3, space="PSUM") as ps_st,
            tc.tile_pool(name="ps_pv", bufs=2, space="PSUM") as ps_pv,
        ):
            # ---- load inputs ----
            x_sb = pp.tile([128, CI, N], bf16)
            for ci in range(CI):
                for ns, nl in NCHUNKS:
                    nc.sync.dma_start(
                        out=x_sb[:, ci, ns : ns + nl],
                        in_=x_d[ci * 128 : (ci + 1) * 128, ns : ns + nl],
                    )
            wq_sb = pp.tile([128, CI, 256], bf16)
            wk_sb = pp.tile([128, CI, 256], bf16)
            wv_sb = pp.tile([128, CI, 256], bf16)
            for w_sb, w_d in ((wq_sb, wqT_d), (wk_sb, wkT_d), (wv_sb, wvT_d)):
                nc.sync.dma_start(
                    out=w_sb[:], in_=w_d.rearrange("(ci p) o -> p ci o", p=128)
                )
            wo_sb = pp.tile([128, 2, 256], f32r)
            nc.sync.dma_start(out=wo_sb[:], in_=woutT_d[:])

            ones_f = pp.tile([128, 1], f32)
            nc.vector.memset(ones_f[:], 1.0)

            # PE warm-up: dummy bf16 matmuls with no input dependencies,
            # executed during the initial DMA wait so the PE p-state is at
            # full speed when the real QKV matmuls arrive.
            warm_sb = pp.tile([128, 512], bf16)
            nc.vector.memset(warm_sb[:], 1.0)
            warm_ps = ps_st.tile([128, 2, 512], f32, tag="st", name="warm_ps")
            for wi in range(32):
                nc.tensor.matmul(
                    warm_ps[:, 0, :],
                    lhsT=warm_sb[:, 0:128],
                    rhs=warm_sb[:],
                    start=(wi == 0),
                    stop=(wi == 31),
                )
            nc.vector.tensor_copy(warm_sb[:, 0:16], warm_ps[:, 0, 0:16])

            def emit_filler(n):
                # dependency-free bf16 matmuls that keep the PE (and its HAM
                # clock state) busy across gaps where it would otherwise wait
                # on another engine
                fps = ps_st.tile([128, 2, 512], f32, tag="st", name="fill_ps")
                for fi in range(n):
                    nc.tensor.matmul(
                        fps[:, 0, :],
                        lhsT=warm_sb[:, 0:128],
                        rhs=warm_sb[:],
                        start=(fi == 0),
                        stop=(fi == n - 1),
                    )

            # vt_sb: [n-part, j-chunk, 4*65]; per head 64 v columns + ones col
            # (filled during the first attention block)
            vt_sb = pp.tile([128, NJ, 260], bf16)
            vt4 = vt_sb.rearrange("p j (h e) -> p j h e", e=65)
            with nc.allow_low_precision(reason="ones column in bf16"):
                nc.vector.tensor_copy(
                    vt4[:, :, :, 64:65],
                    ones_f[:, 0:1]
                    .unsqueeze(1)
                    .unsqueeze(1)
                    .to_broadcast((128, NJ, 4, 1)),
                )

            def emit_vt(j):
                # V^T projection chunk j -> vt_sb (bf16)
                vps3 = ps_st.tile([128, 2, 512], f32, tag="st", name="v_ps")
                vps = vps3[:, 0, 0:256]
                for ci in range(CI):
                    nc.tensor.matmul(
                        vps[:],
                        lhsT=x_sb[:, ci, j * 128 : (j + 1) * 128],
                        rhs=wv_sb[:, ci, :],
                        start=(ci == 0),
                        stop=(ci == CI - 1),
                    )
                with nc.allow_low_precision(reason="v^T stored bf16"):
                    nc.vector.tensor_copy(
                        vt4[:, j, :, 0:64],
                        vps.rearrange("p (h d) -> p h d", h=4),
                    )

            # First half of the V^T projection: keeps the PE warm while the
            # x/w DMAs for q/k drain, and its DVE copies run before the q/k
            # PSUM casts.
            VT_SPLIT = 9
            for j in range(VT_SPLIT):
                emit_vt(j)

            # ---- Q/K projection ----
            # q_sb/k_sb: [d-part, head-pair, n]; heads 2p at part 0-63,
            # 2p+1 at 64-127
            q_sb = pp.tile([128, 2, N], bf16)
            k_sb = pp.tile([128, 2, N], bf16)
            ssq = mp.tile([128, 2, 2, len(NCHUNKS)], f32, tag="ssq")
            scratch = pp.tile([128, 512], f32)
            with nc.allow_low_precision(reason="q/k stored bf16"):
                for ti, (dst, w_sb) in enumerate(((q_sb, wq_sb), (k_sb, wk_sb))):
                    for oc in range(2):
                        for nci, (ns, nl) in enumerate(NCHUNKS):
                            ps3 = ps_st.tile([128, 2, 512], f32, tag="st", name="qk_ps")
                            ps = ps3[:, 0, :]
                            for ci in range(CI):
                                nc.tensor.matmul(
                                    ps[:, :nl],
                                    lhsT=w_sb[:, ci, oc * 128 : (oc + 1) * 128],
                                    rhs=x_sb[:, ci, ns : ns + nl],
                                    start=(ci == 0),
                                    stop=(ci == CI - 1),
                                )
                            nc.vector.tensor_copy(dst[:, oc, ns : ns + nl], ps[:, :nl])
                            nc.scalar.activation(
                                scratch[:, :nl],
                                ps[:, :nl],
                                Square,
                                accum_out=ssq[:, ti, oc, nci : nci + 1],
                            )

            # ---- fold l2norm + SCALE into q: q *= 10/sqrt(ssq_q*ssq_k) ----
            sqk = mp.tile([128, 2, 2], f32, tag="sqk")
            nc.vector.reduce_sum(
                sqk.rearrange("p a b -> p (a b)"),
                ssq.rearrange("p a b c -> p (a b) c"),
                axis=X,
            )
            qscale = mp.tile([128, 2], f32, tag="qscale")
            nc.vector.tensor_tensor(qscale[:], sqk[:, 0, :], sqk[:, 1, :], mult)
            # 10/sqrt(x) = exp(-0.5*ln(x) + ln(10)); Ln and Exp share one ACT
            # table set, so no extra table load next to the softmax exps
            nc.scalar.activation(qscale[:], qscale[:], Ln)
            ln10 = mp.tile([128, 1], f32, tag="ln10")
            nc.vector.memset(ln10[:], 2.302585092994046)
            nc.scalar.activation(qscale[:], qscale[:], Exp, bias=ln10[:], scale=-0.5)

            with nc.allow_low_precision(reason="q scale written as bf16"):
                # chunk-split so the first ST matmuls only wait on chunk 0;
                # emitted BEFORE the remaining vt copies so the in-order DVE
                # unblocks the attention start first
                for ns, nl in NCHUNKS:
                    for oc in range(2):
                        nc.vector.tensor_scalar_mul(
                            q_sb[:, oc, ns : ns + nl],
                            q_sb[:, oc, ns : ns + nl],
                            qscale[:, oc : oc + 1],
                        )

            # Second half of the V^T projection: fills the PE while the
            # l2norm chain and q-scaling run on ACT/DVE, so the PE never
            # idles (an idle PE triggers HAM down-clocking right at the
            # attention start).
            for j in range(VT_SPLIT, NJ):
                emit_vt(j)
            # cover the remaining chain latency before the first ST lands
            emit_filler(10)

            # ---- attention per head pair p (local heads 2p, 2p+1) ----
            # outT_pair[p]: heads 2p / 2p+1 at partitions 0-63 / 64-127, so
            # the output projection contracts K=128 in one matmul per chunk.
            outT = [
                pp.tile([128, N], f32r, name=f"outT{p}", tag=f"outT{p}")
                for p in range(2)
            ]

            def emit_proj(ns, il):
                for oc_ in range(2):
                    yps = ps_pv.tile([128, 512], f32, tag="pv", name="yps")
                    for pr in range(2):
                        nc.tensor.matmul(
                            yps[:, :il],
                            lhsT=wo_sb[:, pr, oc_ * 128 : (oc_ + 1) * 128],
                            rhs=outT[pr][:, ns : ns + il],
                            start=(pr == 0),
                            stop=(pr == 1),
                        )
                    y_sb = mp.tile([128, 512], f32, tag="ysb", name="y_sb")
                    nc.scalar.copy(y_sb[:, :il], yps[:, :il])
                    nc.sync.dma_start(
                        out=y_d[oc_ * 128 : (oc_ + 1) * 128, ns : ns + il],
                        in_=y_sb[:, :il],
                    )

            # Flattened attention stream over blocks (p, nci) with a GLOBAL
            # one-step PV lag: the last PV pair of a block is emitted after
            # the first ST pair of the next block, so the PE never waits for
            # an exp at block boundaries. Each block's PSUM accumulators are
            # allocated lazily on the first PV so only two are ever live.
            class Blk:
                def __init__(self, p, ns, il):
                    self.p, self.ns, self.il = p, ns, il
                    self.pv = None

                def emit_pv(self, pt_, j_):
                    if self.pv is None:
                        self.pv = (
                            ps_pv.tile([65, 512], f32, tag="pv", name="pvA"),
                            ps_pv.tile([65, 512], f32, tag="pv", name="pvB"),
                        )
                    for t in range(2):
                        nc.tensor.matmul(
                            self.pv[t][:, : self.il],
                            lhsT=vt4[:, j_, 2 * self.p + t, :],
                            rhs=pt_[:, t, : self.il],
                            start=(j_ == 0),
                            stop=(j_ == NJ - 1),
                        )

                def emit_tail(self):
                    # Free the PV accumulators into SBUF, then normalize rows
                    # 0-63 by row 64 (softmax denominator): the two [1,il]
                    # denominator rows round-trip through DRAM packed as
                    # [128, 2*il/128] so ONE tiny exact reciprocal covers the
                    # whole block, then partition-replicated back.
                    p, ns, il = self.p, self.ns, self.il
                    nsbs = []
                    for pv in self.pv:
                        nsb = mp.tile([65, 512], f32, tag="nsb", name="nsb", bufs=4)
                        nc.scalar.copy(nsb[:, :il], pv[:, :il])
                        nsbs.append(nsb)
                    den_d = dp.tile([1, 1024], f32, tag="dend", name="den_d")
                    for idx, nsb in enumerate(nsbs):
                        nc.sync.dma_start(
                            out=den_d[0:1, idx * il : (idx + 1) * il],
                            in_=nsb[64:65, :il],
                        )
                    g = (2 * il) // 128
                    denp = mp.tile([128, 8], f32, tag="denp", name="denp", bufs=4)
                    nc.sync.dma_start(
                        out=denp[:, :g],
                        in_=den_d.rearrange("o (pp f) -> (o pp) f", f=g)[0:128, :],
                    )
                    nc.vector.reciprocal(denp[:, :g], denp[:, :g])
                    rec_d = dp.tile([1, 1024], f32, tag="recd", name="rec_d")
                    nc.sync.dma_start(
                        out=rec_d.rearrange("o (pp f) -> (o pp) f", f=g)[0:128, :],
                        in_=denp[:, :g],
                    )
                    for idx, nsb in enumerate(nsbs):
                        bc_sb = mp.tile(
                            [64, 512], f32, tag="bcsb", name="bc_sb", bufs=4
                        )
                        nc.sync.dma_start(
                            out=bc_sb[:, :il],
                            in_=rec_d[0:1, idx * il : (idx + 1) * il].to_broadcast(
                                (64, il)
                            ),
                        )
                        nc.gpsimd.tensor_tensor(
                            outT[p][idx * 64 : (idx + 1) * 64, ns : ns + il],
                            nsb[0:64, :il],
                            bc_sb[:, :il],
                            mult,
                        )

            blocks = [
                Blk(p, ns, il) for p in range(2) for (ns, il) in NCHUNKS
            ]
            pv_q = []  # PV pairs lagging 2 STs behind (exp fully drained)
            pending_proj = None  # (ns, il) deferred one extra block so the
            # proj matmuls never wait on a fresh normalize chain

            def flush_pv(limit):
                nonlocal pending_proj
                while len(pv_q) > limit:
                    b_, pt_, j_ = pv_q.pop(0)
                    b_.emit_pv(pt_, j_)
                    if j_ == NJ - 1:
                        b_.emit_tail()
                        if pending_proj is not None:
                            emit_proj(*pending_proj)
                            pending_proj = None
                        if b_.p == 1:
                            pending_proj = (b_.ns, b_.il)
            for bi, blk in enumerate(blocks):
                p, ns, il = blk.p, blk.ns, blk.il
                for j in range(NJ):
                    st = ps_st.tile([128, 2, 512], f32, tag="st", name="st")
                    nc.tensor.matmul(
                        st[:, 0, :il],
                        lhsT=k_sb[0:64, p, j * 128 : (j + 1) * 128],
                        rhs=q_sb[0:64, p, ns : ns + il],
                    )
                    nc.tensor.matmul(
                        st[:, 1, :il],
                        lhsT=k_sb[64:128, p, j * 128 : (j + 1) * 128],
                        rhs=q_sb[64:128, p, ns : ns + il],
                    )
                    pt = ptp.tile([128, 2, 512], bf16, tag="pt", name="pt")
                    if j % SCHR_MOD == SCHR_MOD - 1:
                        with nc.allow_low_precision(reason="schraudolph exp bits"):
                            nc.vector.tensor_scalar(
                                out=pt[:, :, :il].bitcast(i16),
                                in0=st[:, :, :il],
                                scalar1=A16,
                                scalar2=B16,
                                op0=mult,
                                op1=add,
                            )
                    else:
                        nc.scalar.activation(pt[:, :, :il], st[:, :, :il], Exp)
                    flush_pv(1)
                    pv_q.append((blk, pt, j))

            flush_pv(0)
            # PE filler while the final normalize chain drains on
            # DMA/DVE/GPSIMD: an idle PE triggers HAM down-clocking, which
            # would run the last projection at half speed.
            emit_filler(18)
            if pending_proj is not None:
                emit_proj(*pending_proj)

    return nc


_NC_CACHE = None


def kernel(x, w_qkv, w_out, b_out):
    global _NC_CACHE
    import ml_dtypes
    from concourse.bass_utils import run_bass_kernel_spmd

    bf = ml_dtypes.bfloat16
    x = np.ascontiguousarray(x, dtype=np.float32)
    w_qkv = np.asarray(w_qkv, dtype=np.float32)
    w_out = np.asarray(w_out, dtype=np.float32)
    b_out = np.asarray(b_out, dtype=np.float32)

    b, c, h, w = x.shape
    assert (b, c, h, w) == (B, C, 48, 48)
    x_bn = x.reshape(B, C, N).astype(bf)

    wq, wk, wv = w_qkv[0:HID], w_qkv[HID : 2 * HID], w_qkv[2 * HID : 3 * HID]
    w_outT = np.ascontiguousarray(w_out.T)  # [HID, C]

    in_maps = []
    for core in range(N_CORES):
        bb, g = core // 2, core % 2
        rows = slice(g * 256, g * 256 + 256)
        woutT_c = np.ascontiguousarray(
            w_outT[rows].reshape(2, 128, 256).transpose(1, 0, 2)
        )
        in_maps.append(
            {
                "x": np.ascontiguousarray(x_bn[bb]),
                "wqT": np.ascontiguousarray(wq[rows].T.astype(bf)),
                "wkT": np.ascontiguousarray(wk[rows].T.astype(bf)),
                "wvT": np.ascontiguousarray(wv[rows].T.astype(bf)),
                "woutT": woutT_c,
            }
        )

    if _NC_CACHE is None:
        _NC_CACHE = build_kernel()
    nc = _NC_CACHE

    trace = bool(int(os.environ.get("KERNEL_TRACE", "0")))
    res = run_bass_kernel_spmd(
        nc,
        in_maps,
        core_ids=list(range(N_CORES)),
        trace=trace,
        trace_cores=list(range(N_CORES)) if trace else None,
    )
    kernel.last_result = res

    y = np.empty((B, C, N), dtype=np.float32)
    for bb in range(B):
        y[bb] = (
            res.results[2 * bb]["y"]
            + res.results[2 * bb + 1]["y"]
            + b_out[:, None]
        )
    return y.reshape(B, C, 48, 48)
